# revision 1
# baseline (speedup 1.0000x reference)
"""Trainium2 Bass kernel for nn_Blur (motion-blur via 17-division grid_sample warp).

Sharding: pure data parallel over (batch n, warp half). Core c handles
(n = c//2, side = c%2): side 0 warps X0 with theta_fw, side 1 warps X1 with
theta_bw = theta_fw - flow. Each core returns the W-weighted partial sum over
its 17 divisions; the host adds the two halves, divides by 34 and clips.

Device algorithm per core:
  - The (host-prepared) "shingle" holds, for every possible bilinear patch
    position (j = y0+1, u = x0+1), the 2x2x3ch patch as 12 contiguous f32,
    in 256B-pitch cells interleaved by x-quarter. One dma_gather descriptor
    fetches one sample point's entire patch.
  - Coordinates/weights are computed on-device in a [128=(xh,y64), 256=xm]
    natural layout, PE-transposed to the gather-aligned [x%128, (d,xc,y)]
    layout. Patch indices are written to DRAM in wrapped order and read
    back through the DMA xbar transpose to form the int16 index tensor
    dma_gather requires ([16-partition-wrapped, replicated per queue]).
  - Bilinear interpolation + W-weighted accumulation over d on DVE/ACT,
    final PE un-transpose and DMA out.
"""

import os as _os
import numpy as np

import concourse.bass as bass
import concourse.mybir as mybir
from concourse import bacc
import concourse.tile as tile
from concourse import bass_utils
from concourse.masks import make_identity

F32 = mybir.dt.float32
BF16 = mybir.dt.bfloat16
I32 = mybir.dt.int32
I16 = mybir.dt.int16

S = 512          # image height/width
D = 17           # divisions per side
BLK = 64         # output rows per block
NBLK = S // BLK  # 8
NGRP = BLK // 4  # 16 row-groups (of 4 rows) per block
UQ = 255         # cells per shingle row per x-quarter
PJ = 639         # shingle rows (j in [-63, 575])
ROWSPAN = 127    # shingle rows addressable per gather instruction
CELLSPAN = ROWSPAN * UQ  # 32385 (< 32768, fits int16 non-negative)
NIDX = D * 4 * 128       # 8704 points per gather instruction (d, yg, p)
NSLOT = NIDX // 128      # 68 slots (d*4 + yg)


# --------------------------------------------------------------------------
# relaxed dma_gather (stock helper asserts elem_size_bytes % 256 == 0; the
# ucode only requires the *stride* to be a multiple of 256B)
# --------------------------------------------------------------------------
def _dma_gather_relaxed(nc, out_ap, in_ap, idxs_ap, num_idxs, elem_size,
                        elem_step, queue_num):
    gp = nc.gpsimd
    assert idxs_ap.dtype == I16
    stride_bytes = elem_step * mybir.dt.size(in_ap.dtype)
    stride_bytes_256 = stride_bytes // 256
    assert stride_bytes % 256 == 0 and stride_bytes_256 < 256
    assert in_ap.ap[0][0] == elem_step, in_ap.ap
    assert in_ap.ap[-1][1] == elem_size, in_ap.ap
    assert out_ap.ap[-1][1] == elem_size, out_ap.ap
    _in_ap = gp.lower_ap_dma(in_ap, for_custom_bir_dma=True)
    _idxs_ap = gp.lower_ap(idxs_ap)
    _out_ap = gp.lower_ap(out_ap)
    return gp.add_instruction(
        mybir.InstDMAGatherAnt(
            name=gp.bass.get_next_instruction_name(),
            ins=[*_in_ap, _idxs_ap, gp.lower_val_access(gp.to_reg(num_idxs))],
            outs=[_out_ap],
            transpose=False,
            num_idxs=num_idxs,
            elem_size=elem_size,
            stride_bytes_256=stride_bytes_256,
            gen_mode=0,
            single_packet=False,
            queue_num=queue_num,
            sbuf_tokens_per_rank=0,
            sbuf_free_dim_per_rank=0,
            sbuf_free_dim_pad_per_rank=0,
            sbuf_byte_offset=0,
        )
    )


# --------------------------------------------------------------------------
# device program
# --------------------------------------------------------------------------
def build_program(n_blocks=NBLK, repeat=1):
    nc = bacc.Bacc("TRN2", target_bir_lowering=False, debug=False,
                   num_swdge_queues=4)

    shin = nc.dram_tensor("shin", [PJ * UQ * 64], F32, kind="ExternalInput")
    sx = nc.dram_tensor("sx", [D, S, S], F32, kind="ExternalInput")
    sy = nc.dram_tensor("sy", [D, S, S], F32, kind="ExternalInput")
    Wt = nc.dram_tensor("Wt", [D, S, S], F32, kind="ExternalInput")
    fxd = nc.dram_tensor("fxd", [S, S], F32, kind="ExternalInput")
    fyd = nc.dram_tensor("fyd", [S, S], F32, kind="ExternalInput")
    csini = nc.dram_tensor("csini", [128, 1], F32, kind="ExternalInput")
    cgx = nc.dram_tensor("cgx", [128, 256], F32, kind="ExternalInput")
    cxm = nc.dram_tensor("cxm", [128, 256], F32, kind="ExternalInput")
    cgy = nc.dram_tensor("cgy", [NBLK * 128, 1], F32, kind="ExternalInput")
    cpart = nc.dram_tensor("cpart", [NBLK * 128, 1], F32, kind="ExternalInput")
    out = nc.dram_tensor("out", [3, S, S], F32, kind="ExternalOutput")
    mbuf = nc.dram_tensor("mbuf", [n_blocks, NGRP, 4, NIDX // 16, 128], I16,
                          kind="Internal")

    shin2d = shin.rearrange("(n e) -> n e", e=64)
    MUL = mybir.AluOpType.mult
    ADD = mybir.AluOpType.add
    SUB = mybir.AluOpType.subtract
    MAX = mybir.AluOpType.max
    MIN = mybir.AluOpType.min

    with tile.TileContext(nc) as tc:
        with (
            tc.tile_pool(name="const", bufs=1) as cpool,
            tc.tile_pool(name="store", bufs=2) as spool,
            tc.tile_pool(name="work", bufs=2) as wpool,
            tc.tile_pool(name="gath", bufs=2) as gpool,
            tc.tile_pool(name="gatd", bufs=3) as gdpool,
            tc.tile_pool(name="widxp", bufs=5) as wxpool,
            tc.tile_pool(name="psum", bufs=2, space="PSUM") as ppool,
            tc.tile_pool(name="psum2", bufs=2, space="PSUM") as ppool2,
        ):
            # persistent constants
            ident = cpool.tile([128, 128], F32)
            make_identity(nc, ident[:])
            cgx_t = cpool.tile([128, 256], F32)
            nc.sync.dma_start(out=cgx_t[:], in_=cgx[:])
            cxm_t = cpool.tile([128, 256], F32)
            nc.sync.dma_start(out=cxm_t[:], in_=cxm[:])
            csini_t = cpool.tile([128, 1], F32)
            nc.sync.dma_start(out=csini_t[:], in_=csini[:])

            def coords_phase(blk):
                ys, ye = blk * BLK, (blk + 1) * BLK

                cgy_t = wpool.tile([128, 1], F32, tag="cgy")
                nc.sync.dma_start(out=cgy_t[:], in_=cgy[blk * 128:(blk + 1) * 128, :])
                cpart_t = wpool.tile([128, 1], F32, tag="cpart")
                nc.sync.dma_start(out=cpart_t[:],
                                  in_=cpart[blk * 128:(blk + 1) * 128, :])

                # flow rows for this block: [128=(xh,y), 256] per component
                fx_t = wpool.tile([128, 256], F32, tag="fx")
                fy_t = wpool.tile([128, 256], F32, tag="fy")
                nc.sync.dma_start(
                    out=fx_t[:],
                    in_=fxd[ys:ye, :].rearrange("y (h m) -> h y m", h=2))
                nc.sync.dma_start(
                    out=fy_t[:],
                    in_=fyd[ys:ye, :].rearrange("y (h m) -> h y m", h=2))

                # running cumsums
                csx_t = wpool.tile([128, 256], F32, tag="csx")
                csy_t = wpool.tile([128, 256], F32, tag="csy")

                # per-block stores (gather-aligned [x%128, (d, xc, yb)] layout)
                st_a = spool.tile([128, D, 4, BLK], BF16, tag="st_a")
                st_b = spool.tile([128, D, 4, BLK], BF16, tag="st_b")
                st_w = spool.tile([128, D, 4, BLK], BF16, tag="st_w")
                idx16 = spool.tile([128, 2, D, 8, 16], I16, tag="idx16")

                acc = spool.tile([128, 3, 4, BLK], F32, tag="acc")
                nc.vector.memset(acc[:], 0.0)

                for d in range(D):
                    sx_t = wpool.tile([128, 256], F32, tag="sx")
                    sy_t = wpool.tile([128, 256], F32, tag="sy")
                    w_t = wpool.tile([128, 256], F32, tag="w")
                    rsx = sx[d, ys:ye, :].rearrange("y (h m) -> h y m", h=2)
                    rsy = sy[d, ys:ye, :].rearrange("y (h m) -> h y m", h=2)
                    rw = Wt[d, ys:ye, :].rearrange("y (h m) -> h y m", h=2)
                    nc.sync.dma_start(out=sx_t[:], in_=rsx)
                    nc.sync.dma_start(out=sy_t[:], in_=rsy)
                    nc.sync.dma_start(out=w_t[:], in_=rw)

                    if d == 0:
                        nc.vector.tensor_scalar(out=csx_t[:], in0=sx_t[:],
                                                scalar1=csini_t[:, :1],
                                                scalar2=None, op0=ADD)
                        nc.vector.tensor_scalar(out=csy_t[:], in0=sy_t[:],
                                                scalar1=csini_t[:, :1],
                                                scalar2=None, op0=ADD)
                    else:
                        nc.vector.tensor_tensor(out=csx_t[:], in0=csx_t[:],
                                                in1=sx_t[:], op=ADD)
                        nc.vector.tensor_tensor(out=csy_t[:], in0=csy_t[:],
                                                in1=sy_t[:], op=ADD)

                    # ---- coordinates ----
                    tx = wpool.tile([128, 256], F32, tag="tx")
                    ty = wpool.tile([128, 256], F32, tag="ty")
                    nc.vector.tensor_tensor(out=tx[:], in0=csx_t[:], in1=fx_t[:], op=MUL)
                    nc.vector.tensor_tensor(out=ty[:], in0=csy_t[:], in1=fy_t[:], op=MUL)
                    # gx = (x - tx - 256)/256 = tx*(-1/256) + cgx ; clip to [-1,1]
                    gx = wpool.tile([128, 256], F32, tag="gx")
                    gy = wpool.tile([128, 256], F32, tag="gy")
                    nc.vector.scalar_tensor_tensor(out=gx[:], in0=tx[:],
                                                   scalar=-1.0 / 256.0, in1=cgx_t[:],
                                                   op0=MUL, op1=ADD)
                    nc.vector.tensor_scalar(out=gy[:], in0=ty[:],
                                            scalar1=-1.0 / 256.0,
                                            scalar2=cgy_t[:, :1],
                                            op0=MUL, op1=ADD)
                    nc.vector.tensor_scalar(out=gx[:], in0=gx[:], scalar1=-1.0,
                                            scalar2=1.0, op0=MAX, op1=MIN)
                    nc.vector.tensor_scalar(out=gy[:], in0=gy[:], scalar1=-1.0,
                                            scalar2=1.0, op0=MAX, op1=MIN)
                    # t = ix + 256 = gx*256 + 511.5 ; u0 = int-convert(t)
                    # (trunc -> x0+256 exactly; RNE -> x0+256 or +257)
                    # correction: e = t - u0; if e < 0: u0 -= 1, e += 1
                    # => u_f = x0 + 256 exactly, wx1 = e in [0, 1)
                    tpx = wpool.tile([128, 256], F32, tag="tpx")
                    tpy = wpool.tile([128, 256], F32, tag="tpy")
                    nc.vector.tensor_scalar(out=tpx[:], in0=gx[:], scalar1=256.0,
                                            scalar2=511.5, op0=MUL, op1=ADD)
                    nc.vector.tensor_scalar(out=tpy[:], in0=gy[:], scalar1=256.0,
                                            scalar2=511.5, op0=MUL, op1=ADD)
                    u_i = wpool.tile([128, 256], I32, tag="u_i")
                    j_i = wpool.tile([128, 256], I32, tag="j_i")
                    nc.vector.tensor_copy(out=u_i[:], in_=tpx[:])
                    nc.vector.tensor_copy(out=j_i[:], in_=tpy[:])
                    u_f = wpool.tile([128, 256], F32, tag="u_f")
                    j_f = wpool.tile([128, 256], F32, tag="j_f")
                    nc.vector.tensor_copy(out=u_f[:], in_=u_i[:])
                    nc.vector.tensor_copy(out=j_f[:], in_=j_i[:])
                    ex = wpool.tile([128, 256], F32, tag="ex")
                    ey = wpool.tile([128, 256], F32, tag="ey")
                    nc.vector.tensor_tensor(out=ex[:], in0=tpx[:], in1=u_f[:], op=SUB)
                    nc.vector.tensor_tensor(out=ey[:], in0=tpy[:], in1=j_f[:], op=SUB)
                    ltx = wpool.tile([128, 256], F32, tag="ltx")
                    lty = wpool.tile([128, 256], F32, tag="lty")
                    nc.vector.tensor_scalar(out=ltx[:], in0=ex[:], scalar1=0.0,
                                            scalar2=None,
                                            op0=mybir.AluOpType.is_lt)
                    nc.vector.tensor_scalar(out=lty[:], in0=ey[:], scalar1=0.0,
                                            scalar2=None,
                                            op0=mybir.AluOpType.is_lt)
                    nc.vector.tensor_tensor(out=u_f[:], in0=u_f[:], in1=ltx[:], op=SUB)
                    nc.vector.tensor_tensor(out=j_f[:], in0=j_f[:], in1=lty[:], op=SUB)
                    wx1 = wpool.tile([128, 256], F32, tag="wx1")
                    wy1 = wpool.tile([128, 256], F32, tag="wy1")
                    nc.vector.tensor_tensor(out=wx1[:], in0=ex[:], in1=ltx[:], op=ADD)
                    nc.vector.tensor_tensor(out=wy1[:], in0=ey[:], in1=lty[:], op=ADD)
                    # a = W*(1-wy1), b = W*wy1
                    bt = wpool.tile([128, 256], F32, tag="bt")
                    at = wpool.tile([128, 256], F32, tag="at")
                    nc.vector.tensor_tensor(out=bt[:], in0=w_t[:], in1=wy1[:], op=MUL)
                    nc.vector.tensor_tensor(out=at[:], in0=w_t[:], in1=bt[:], op=SUB)
                    # idx = j256*255 + cpart + u256 + cxm
                    s1 = wpool.tile([128, 256], F32, tag="s1")
                    nc.vector.tensor_scalar(out=s1[:], in0=j_f[:], scalar1=255.0,
                                            scalar2=cpart_t[:, :1], op0=MUL, op1=ADD)
                    nc.vector.tensor_tensor(out=s1[:], in0=s1[:], in1=u_f[:], op=ADD)
                    nc.vector.tensor_tensor(out=s1[:], in0=s1[:], in1=cxm_t[:], op=ADD)
                    nc.vector.tensor_copy(
                        out=idx16[:, :, d, :, :],
                        in_=s1[:].rearrange("p (h p16 pp) -> p h p16 pp", h=2, p16=8))

                    # ---- PE transposes into gather-aligned stores ----
                    # one full [128,128] transpose per (tensor, xm-half):
                    # out[xm, (xh, yb)]; xm = x%128 for both xh at once.
                    _tlist = () if int(_os.environ.get("STUB", "0")) >= 5 else (
                        (at, st_a), (bt, st_b), (wx1, st_w))
                    for (src, dst) in _tlist:
                        for half in range(2):
                            pt = ppool.tile([128, 128], F32, tag="pt")
                            nc.tensor.transpose(
                                out=pt[:],
                                in_=src[:, 128 * half:128 * half + 128],
                                identity=ident[:],
                            )
                            # pt[:, xh*64:+64] -> dst[:, d, 2*xh+half, :]
                            nc.vector.tensor_copy(
                                out=dst[:, d, half::2, :],
                                in_=pt[:].rearrange("p (xh y) -> p xh y", xh=2))

                # ---- M-stream writes (wrapped idx order) ----
                # mbuf[blk, g, xc, f, c]: f = (yg*17+d)*8 + p16, c = 16*rep + pp
                if int(_os.environ.get("STUB", "0")) < 4:
                    for xc in range(4):
                        xh, half = xc // 2, xc % 2
                        for yg in range(4):
                            src_v = idx16[64 * xh + yg:64 * xh + 64:4,
                                          half].rearrange(
                                              "p d p16 pp -> p (d p16) pp")
                            dst_v = mbuf[blk, :, xc][:, 136 * yg:136 * (yg + 1),
                                                     0:16]
                            nc.sync.dma_start(out=dst_v, in_=src_v)
                    # replicate cols [0:16] -> [16:128] (log doubling); the
                    # gather ucode reads per-engine idx replicas.
                    for w in (16, 32, 64):
                        nc.sync.dma_start(out=mbuf[blk][:, :, :, w:2 * w],
                                          in_=mbuf[blk][:, :, :, 0:w])
                return st_a, st_b, st_w, idx16, acc

            def gath_phase(blk, st_a, st_b, st_w, idx16, acc):
                ys, ye = blk * BLK, (blk + 1) * BLK
                # ---- per-group: xbar transpose idx, gather, interp ----
                for g in range(NGRP if int(_os.environ.get("STUB", "0")) < 4 else 0):
                    G = blk * NGRP + g
                    gat = gdpool.tile([128, 4, NSLOT, 12], F32, tag="gat")
                    _stub = int(_os.environ.get("STUB", "0"))
                    if _stub >= 2:
                        nc.vector.memset(gat[:], 0.25)
                    widx = wxpool.tile([128, 4, NIDX // 16], I16, tag="widx")
                    nc.sync.dma_start(
                        out=widx[:],
                        in_=mbuf[blk, g].rearrange("xc f c -> (xc f) c"),
                        transpose=True)
                    for xc in range(4):
                        if _stub >= 2:
                            continue
                        cell0 = (4 * G + 2) * UQ
                        in_ap = shin2d[cell0:cell0 + CELLSPAN,
                                       xc * 16:xc * 16 + 12]
                        _dma_gather_relaxed(
                            nc, out_ap=gat[:, xc], in_ap=in_ap, idxs_ap=widx[:, xc],
                            num_idxs=NIDX, elem_size=12, elem_step=64,
                            queue_num=_stub and 0 or xc)

                    if int(_os.environ.get("STUB", "0")) >= 3:
                        continue
                    # interp: coeffs on [128, (xc, d, yg)] = [128, 272]
                    def wview(st):
                        return st[:, :, :, 4 * g:4 * g + 4].rearrange(
                            "p d x y -> p x y d")
                    a_v, b_v, w_v = wview(st_a), wview(st_b), wview(st_w)
                    c01 = gpool.tile([128, 4, 4, D], F32, tag="c01")
                    c00 = gpool.tile([128, 4, 4, D], F32, tag="c00")
                    c11 = gpool.tile([128, 4, 4, D], F32, tag="c11")
                    c10 = gpool.tile([128, 4, 4, D], F32, tag="c10")
                    nc.vector.tensor_tensor(out=c01[:], in0=a_v, in1=w_v, op=MUL)
                    nc.vector.tensor_tensor(out=c00[:], in0=a_v, in1=c01[:], op=SUB)
                    nc.vector.tensor_tensor(out=c11[:], in0=b_v, in1=w_v, op=MUL)
                    nc.vector.tensor_tensor(out=c10[:], in0=b_v, in1=c11[:], op=SUB)

                    for ch in range(3):
                        def gview(tap):
                            return gat[:, :, :, 4 * ch + tap].rearrange(
                                "p x (y d) -> p x y d", y=4)
                        t0 = gpool.tile([128, 4, 4, D], F32, tag="t0")
                        t1 = gpool.tile([128, 4, 4, D], F32, tag="t1")
                        nc.vector.tensor_tensor(out=t0[:], in0=c00[:],
                                                in1=gview(0), op=MUL)
                        nc.vector.tensor_tensor(out=t1[:], in0=c01[:],
                                                in1=gview(1), op=MUL)
                        nc.vector.tensor_tensor(out=t0[:], in0=t0[:], in1=t1[:], op=ADD)
                        nc.vector.tensor_tensor(out=t1[:], in0=c10[:],
                                                in1=gview(2), op=MUL)
                        nc.vector.tensor_tensor(out=t0[:], in0=t0[:], in1=t1[:], op=ADD)
                        nc.vector.tensor_tensor(out=t1[:], in0=c11[:],
                                                in1=gview(3), op=MUL)
                        nc.vector.tensor_tensor(out=t0[:], in0=t0[:], in1=t1[:], op=ADD)
                        # reduce over d: [128, xc, d, yg] -> [128, xc, yg]
                        red = gpool.tile([128, 4, 4], F32, tag="red")
                        nc.vector.tensor_reduce(
                            out=red[:], in_=t0[:],
                            axis=mybir.AxisListType.X, op=ADD)
                        accv = acc[:, ch, :, 4 * g:4 * g + 4]
                        nc.vector.tensor_tensor(out=accv, in0=accv, in1=red[:], op=ADD)

                # ---- un-transpose acc and write out ----
                for ch in range(3):
                    ost = wpool.tile([64, S], F32, tag="ost")
                    for xc in range(4):
                        po = ppool2.tile([64, 128], F32, tag="po")
                        nc.tensor.transpose(out=po[:], in_=acc[:, ch, xc, :],
                                            identity=ident[:])
                        nc.scalar.copy(out=ost[:, 128 * xc:128 * (xc + 1)], in_=po[:])
                    nc.sync.dma_start(out=out[ch, ys:ye, :], in_=ost[:])

            # Software pipeline by one block: emit block b's gather/interp
            # phase after block b+1's coordinate phase, so the in-order DVE
            # stream runs coords(b+1) while Pool gathers block b.
            for _rep in range(repeat):
                prev = None
                for blk in range(n_blocks):
                    cur = (blk, coords_phase(blk))
                    if prev is not None:
                        gath_phase(prev[0], *prev[1])
                    prev = cur
                gath_phase(prev[0], *prev[1])

    nc.compile()
    return nc


# --------------------------------------------------------------------------
# host side
# --------------------------------------------------------------------------
def _build_shingle(img):
    """img [3, 512, 512] -> flat f32 [PJ*UQ*64] quarter-interleaved shingle."""
    Ppad = np.zeros((3, 514, 640), np.float32)
    Ppad[:, 1:513, 64:576] = img
    Sh = np.zeros((PJ, UQ, 4, 16), np.float32)
    rows = np.arange(63, 576)
    cols = (np.arange(4) * 128)[None, :] + np.arange(UQ)[:, None]  # [UQ, 4]
    for ch in range(3):
        for r in range(2):
            for c in range(2):
                Sh[63:576, :, :, ch * 4 + r * 2 + c] = \
                    Ppad[ch][(rows - 63 + r)][:, cols + c]
    return Sh.reshape(-1)


def _consts():
    p = np.arange(128)
    xm = np.arange(256)
    cgx = (((p // 64) * 256)[:, None] + xm[None, :] - 256).astype(np.float32) / 256.0
    cxm = np.broadcast_to((-128.0 * (xm // 128)).astype(np.float32),
                          (128, 256)).copy()
    y_all = (np.arange(NBLK * 128) // 128) * 64 + (np.arange(NBLK * 128) % 128) % 64
    cgy = ((y_all - 256).astype(np.float32) / 256.0)[:, None]
    xh_all = (np.arange(NBLK * 128) % 128) // 64
    cpart = (-255.0 * (194 + 4 * (y_all // 4)) - 192.0 - 256.0 * xh_all
             ).astype(np.float32)[:, None]
    return cgx, cxm, cgy, cpart


def kernel(**inputs):
    X0 = np.asarray(inputs["X0"], np.float32)
    X1 = np.asarray(inputs["X1"], np.float32)
    W = np.asarray(inputs["W"], np.float32)
    sx = np.asarray(inputs["sx"], np.float32)
    sy = np.asarray(inputs["sy"], np.float32)
    flow = np.asarray(inputs["flow"], np.float32)
    N = X0.shape[0]

    n_blocks = int(_os.environ.get("BLUR_BLOCKS", str(NBLK)))
    cgx, cxm, cgy, cpart = _consts()
    in_maps = []
    for c in range(2 * N):
        n, side = c // 2, c % 2
        img = X0[n] if side == 0 else X1[n]
        in_maps.append({
            "shin": _build_shingle(img),
            "sx": sx[n], "sy": sy[n],
            "Wt": W[n, side * D:(side + 1) * D],
            "fxd": np.ascontiguousarray(flow[n, :, :, 0]),
            "fyd": np.ascontiguousarray(flow[n, :, :, 1]),
            "csini": np.full((128, 1), -1.0 * side, np.float32),
            "cgx": cgx, "cxm": cxm, "cgy": cgy, "cpart": cpart,
        })

    nc = build_program(n_blocks=n_blocks)
    res = bass_utils.run_bass_kernel_spmd(nc, in_maps, core_ids=list(range(2 * N)),
                                          trace=bool(_os.environ.get("BLUR_TRACE")))
    if res.exec_time_ns is not None:
        print(f"HW exec time: {res.exec_time_ns} ns")
    kernel.last_results = res
    outs = [r["out"] for r in res.results]
    full = np.stack([
        np.clip((outs[2 * n] + outs[2 * n + 1]) / (2.0 * D), 0.0, 255.0)
        for n in range(N)
    ]).astype(np.float32)
    return full

